# revision 1
# baseline (speedup 1.0000x reference)
"""BiDAF attention-flow kernel for Trainium2 (8 NeuronCores, data-parallel over batch).

Per core (one batch element):
  s[j,i]   = c[j] + q[i] + sum_h w_cq[h]*emb2[j,h]*emb1[i,h]
  a        = softmax_i(s)          (c[j] drops out of the row softmax)
  y2x      = a @ emb1
  b_att    = softmax_j(max_i s)
  x2y      = sum_j b_att[j]*emb2[j]
  out      = [emb2, y2x, emb2*y2x, emb2*x2y] @ w_red + b_red

Implementation notes:
  - b_c/b_q/b_cq cancel exactly in both softmaxes (row/column constants).
  - The row softmax uses a FIXED exp shift (s - SHIFT) instead of a row max:
    softmax is shift-invariant and fp32/bf16 exponent range absorbs the
    spread.  The true row max (needed for b_att) is recovered afterwards as
    SHIFT + ln(max_i u).  This removes the matmul->rowmax->exp serial chain.
  - y2x normalization (1/Z_j) is applied to the *output* psum of the
    reduction matmul blocks 2,3 (per-partition scalar in natural layout).
  - block1 + block4 = emb2 @ (w1 + x2y*w4): merged into one pass-2 matmul.
  - All bf16 transposes (emb1^T, emb2^T, u^T) run on the DMA transpose
    crossbar, keeping PE free for matmuls.
"""

import numpy as np
import ml_dtypes

P = 128
XL = 2048
YL = 2048
H = 768
OUT = 300
NJT = YL // P   # 16 j tiles
NIC = XL // P   # 16 i chunks
NHC = H // P    # 6 h chunks
SLAB = 512
NSLAB = XL // SLAB  # 4
NCORES = 8
SHIFT = 10.0    # fixed exp shift; |s| stays well below this + fp range

_CACHE = {}


def _fix_waits(nc, mybir, max_waits=1):
    """This walrus build rejects >1 sync wait per instruction.

    Pass 1: drop waits that are transitively implied by another wait on the
    same instruction (happens-before over per-engine / per-DMA-queue in-order
    streams plus wait edges).  Pass 2: hoist remaining extra waits onto
    same-engine NoOps inserted right before the instruction (for an in-order
    engine this blocks identically; DMA triggers are all on SP here and their
    awaited DMAs are always triggered earlier, so no cycles arise).
    """
    from collections import defaultdict

    blocks = [bb for f in nc.m.functions for bb in f.blocks]
    insts = [ins for bb in blocks for ins in bb.instructions]

    dma_types = ("InstDMACopy", "InstDmaTransposeAnt")
    eng_stream = defaultdict(list)
    queue_stream = defaultdict(list)
    sem_events = defaultdict(list)
    cum = defaultdict(int)
    for i, ins in enumerate(insts):
        eng_stream[str(ins.engine)].append(i)
        si = ins.sync_info
        if si and si.on_update:
            for u in si.on_update:
                cum[u.id] += u.update_value
                sem_events[u.id].append((cum[u.id], i))
                if type(ins).__name__ in dma_types:
                    queue_stream[u.id].append(i)

    def achiever(sem_id, val):
        for cv, i in sem_events.get(sem_id, []):
            if cv >= val:
                return i
        return None

    eng_pos, q_pos = {}, {}
    for e, lst in eng_stream.items():
        for k, i in enumerate(lst):
            eng_pos[i] = (e, k)
    for s, lst in queue_stream.items():
        for k, i in enumerate(lst):
            q_pos[i] = (s, k)

    memo = {}

    def implied(i):
        if i in memo:
            return memo[i]
        memo[i] = set()
        out = {i}
        ins = insts[i]
        if i in q_pos:
            s, k = q_pos[i]
            if k > 0:
                out |= implied(queue_stream[s][k - 1])
        e, k = eng_pos[i]
        j = k - 1
        while j >= 0:
            p = eng_stream[e][j]
            if type(insts[p]).__name__ in dma_types:
                j -= 1
                continue
            out |= implied(p)
            break
        si = ins.sync_info
        if si and si.on_wait:
            for w in si.on_wait:
                a = achiever(w.id, w.wait_value)
                if a is not None:
                    out |= implied(a)
        memo[i] = out
        return out

    # pass 1: redundancy elimination
    for i, ins in enumerate(insts):
        si = ins.sync_info
        if not (si and si.on_wait and len(si.on_wait) > max_waits):
            continue
        waits = list(si.on_wait)
        ach = [(w, achiever(w.id, w.wait_value)) for w in waits]
        keep = []
        for wi, (w, a) in enumerate(ach):
            red = False
            if a is not None:
                for wj, (w2, a2) in enumerate(ach):
                    if wi != wj and a2 is not None and a != a2 and a in implied(a2):
                        red = True
                        break
            if not red:
                keep.append(w)
        si.on_wait = keep

    # pass 2: hoist extras onto same-engine NoOps
    k = 0
    for bb in blocks:
        lst = bb.instructions
        i = 0
        while i < len(lst):
            ins = lst[i]
            si = ins.sync_info
            if si and si.on_wait and len(si.on_wait) > max_waits:
                waits = list(si.on_wait)
                extra, keep = waits[:-max_waits], waits[-max_waits:]
                si.on_wait = keep
                nops = []
                for w in extra:
                    nop = mybir.InstNoOp(name=f"I-waitfix-{k}", ins=[], outs=[])
                    k += 1
                    nop.engine = ins.engine
                    nop.sync_info = mybir.SyncInfo(on_wait=[w], on_update=[])
                    nops.append(nop)
                lst[i:i] = nops
                i += len(nops)
            i += 1


def _build():
    import concourse.bass as bass
    import concourse.tile as tile
    import concourse.mybir as mybir
    from concourse.masks import make_identity

    f32 = mybir.dt.float32
    f32r = mybir.dt.float32r
    bf16 = mybir.dt.bfloat16
    MUL = mybir.AluOpType.mult
    ADD = mybir.AluOpType.add
    MAX = mybir.AluOpType.max
    EXP = mybir.ActivationFunctionType.Exp
    LN = mybir.ActivationFunctionType.Ln
    AXX = mybir.AxisListType.X

    nc = bass.Bass("TRN2", target_bir_lowering=False, debug=False,
                   num_devices=NCORES)

    emb1_d = nc.dram_tensor("emb1", [XL, H], bf16, kind="ExternalInput")
    emb2_d = nc.dram_tensor("emb2", [YL, H], bf16, kind="ExternalInput")
    wc_d = nc.dram_tensor("wc", [P, NHC], bf16, kind="ExternalInput")
    wq_d = nc.dram_tensor("wq", [P, NHC], bf16, kind="ExternalInput")
    wcq_d = nc.dram_tensor("wcq", [P, NHC], f32, kind="ExternalInput")
    w1_d = nc.dram_tensor("w1", [H, OUT], f32, kind="ExternalInput")
    w2_d = nc.dram_tensor("w2", [H, OUT], bf16, kind="ExternalInput")
    w3_d = nc.dram_tensor("w3", [H, OUT], bf16, kind="ExternalInput")
    w4_d = nc.dram_tensor("w4", [H, OUT], f32, kind="ExternalInput")
    bred_d = nc.dram_tensor("bred", [1, OUT], f32, kind="ExternalInput")
    out_d = nc.dram_tensor("out", [YL, OUT], f32, kind="ExternalOutput")

    with tile.TileContext(nc) as tc:
        with (
            tc.tile_pool(name="res", bufs=1) as res,        # resident data
            tc.tile_pool(name="stage", bufs=3) as stage,    # dma staging
            tc.tile_pool(name="small", bufs=1) as small,    # stats etc
            tc.tile_pool(name="pst", bufs=2, space="PSUM") as pst,
            tc.tile_pool(name="pss", bufs=2, space="PSUM") as pss,
            tc.tile_pool(name="psy", bufs=1, space="PSUM") as psy,
            tc.tile_pool(name="pso", bufs=2, space="PSUM") as pso,
            tc.tile_pool(name="dpool", bufs=1, space="DRAM") as dpool,
        ):
            # ---- constants / weights ----
            ident16 = res.tile([P, P], bf16, tag="ident16")
            make_identity(nc, ident16)
            ident32 = res.tile([P, P], f32, tag="ident32")
            make_identity(nc, ident32)
            ones32 = res.tile([1, P], f32, tag="ones32")
            nc.vector.memset(ones32, 1.0)
            ones_r = res.tile([1, P], f32r, tag="ones_r")
            nc.vector.tensor_copy(out=ones_r, in_=ones32)
            negC = res.tile([P, 1], f32, tag="negC")
            nc.vector.memset(negC, -SHIFT)

            # PE warm-up: keep the HAM activity monitor busy while the input
            # DMAs stream in, so the clock is at 2.4 GHz when the real
            # matmuls start.  No data deps; results are discarded.
            for wk in range(220):
                wps = pss.tile([P, P], bf16, tag="pss", name=f"warm{wk}")
                nc.tensor.transpose(wps, ident16, ident16)

            wc_sb = res.tile([P, NHC], bf16, tag="wc")
            nc.sync.dma_start(out=wc_sb, in_=wc_d[:])
            wq_sb = res.tile([P, NHC], bf16, tag="wq")
            nc.sync.dma_start(out=wq_sb, in_=wq_d[:])
            wcq_sb = res.tile([P, NHC], f32, tag="wcq")
            nc.sync.dma_start(out=wcq_sb, in_=wcq_d[:])

            w1_sb = res.tile([P, NHC, OUT], f32, tag="w1")
            w2_sb = res.tile([P, NHC, OUT], bf16, tag="w2")
            w3_sb = res.tile([P, NHC, OUT], bf16, tag="w3")
            w4_sb = res.tile([P, NHC, OUT], f32, tag="w4")
            for hc in range(NHC):
                nc.sync.dma_start(out=w1_sb[:, hc, :], in_=w1_d[hc * P:(hc + 1) * P, :])
                nc.sync.dma_start(out=w2_sb[:, hc, :], in_=w2_d[hc * P:(hc + 1) * P, :])
                nc.sync.dma_start(out=w3_sb[:, hc, :], in_=w3_d[hc * P:(hc + 1) * P, :])
                nc.sync.dma_start(out=w4_sb[:, hc, :], in_=w4_d[hc * P:(hc + 1) * P, :])
            bred_bc = res.tile([P, OUT], f32, tag="bred_bc")
            _bap = bred_d.ap()
            nc.sync.dma_start(out=bred_bc, in_=bass.AP(
                tensor=_bap.tensor, offset=_bap.offset,
                ap=[[0, P]] + list(_bap.ap[1:])))

            # ---- resident embeddings ----
            # e1n: natural (i, h) bf16; e1tt: emb1^T as (h_in, hc, i) bf16
            # e2tt: emb2^T as (h_in, hc, j) bf16; e2ts: e2tt * w_cq
            e1n = [res.tile([P, H], bf16, tag=f"e1n{i}", name=f"e1n{i}")
                   for i in range(NIC)]
            e1tt = res.tile([P, NHC, XL], bf16, tag="e1tt")
            e2tt = res.tile([P, NHC, YL], bf16, tag="e2tt")
            e2ts = res.tile([P, NHC, YL], bf16, tag="e2ts")

            def load_e2_chunk(jc):
                jsl = slice(jc * P, (jc + 1) * P)
                st = stage.tile([P, H], bf16, tag="stage", name=f"e2st{jc}")
                nc.sync.dma_start(out=st, in_=emb2_d[jsl, :])
                for hc in range(NHC):
                    ps = pst.tile([P, P], bf16, tag="pst", name=f"e2ps{jc}_{hc}")
                    nc.tensor.transpose(ps, st[:, hc * P:(hc + 1) * P], ident16)
                    nc.any.tensor_copy(out=e2tt[:, hc, jsl], in_=ps)
                    nc.vector.tensor_scalar_mul(
                        e2ts[:, hc, jsl], ps, wcq_sb[:, hc:hc + 1])

            def load_e1_chunk(ic):
                isl = slice(ic * P, (ic + 1) * P)
                nc.sync.dma_start(out=e1n[ic], in_=emb1_d[isl, :])
                for hc in range(NHC):
                    ps = pst.tile([P, P], bf16, tag="pst", name=f"e1ps{ic}_{hc}")
                    nc.tensor.transpose(ps, e1n[ic][:, hc * P:(hc + 1) * P],
                                        ident16)
                    nc.any.tensor_copy(out=e1tt[:, hc, isl], in_=ps)

            load_e2_chunk(0)
            for ic in range(NIC):
                load_e1_chunk(ic)
            for jc in range(1, NJT):
                load_e2_chunk(jc)

            # ---- q_row = emb1 @ w_q as a (1, XL) row ----
            q_row = small.tile([1, XL], f32, tag="q_row")
            for sl in range(NSLAB):
                ssl = slice(sl * SLAB, (sl + 1) * SLAB)
                qp = pst.tile([1, SLAB], f32, tag="pst", name=f"qp{sl}")
                for hc in range(NHC):
                    nc.tensor.matmul(
                        qp, wq_sb[:, hc:hc + 1], e1tt[:, hc, ssl],
                        start=(hc == 0), stop=(hc == NHC - 1),
                        skip_group_check=True)
                nc.any.tensor_copy(out=q_row[:, ssl].bitcast(f32r), in_=qp)

            # c_row = emb2 @ w_c as a (1, YL) row (wc stationary), then
            # reshape to (P, NJT) columns via a DRAM bounce
            c_rowf = small.tile([1, YL], f32, tag="c_rowf")
            for sl in range(NSLAB):
                ssl = slice(sl * SLAB, (sl + 1) * SLAB)
                cp = pst.tile([1, SLAB], f32, tag="pst", name=f"cp{sl}")
                for hc in range(NHC):
                    nc.tensor.matmul(
                        cp, wc_sb[:, hc:hc + 1], e2tt[:, hc, ssl],
                        start=(hc == 0), stop=(hc == NHC - 1),
                        skip_group_check=True)
                nc.any.tensor_copy(out=c_rowf[:, ssl], in_=cp)
            crd = dpool.tile([1, YL], f32, tag="crd")
            nc.sync.dma_start(out=crd, in_=c_rowf)
            c_sb = small.tile([P, NJT], f32, tag="c_sb")
            nc.sync.dma_start(out=c_sb, in_=bass.AP(
                tensor=crd.tensor, offset=crd.offset, ap=[[1, P], [P, NJT]]))

            # ---- stats tiles ----
            M_sb = small.tile([P, NJT], f32, tag="M")
            Z_sb = small.tile([P, NJT], f32, tag="Z")
            rZ_sb = small.tile([P, NJT], f32, tag="rZ")
            out_sb = res.tile([P, NJT, OUT], f32, tag="out_sb")

            # ---- main loop over j tiles ----
            sjt_cm = tc.tile_pool(name="sjt", bufs=2)
            sjt = sjt_cm.__enter__()
            for jt in range(NJT):
                jsl = slice(jt * P, (jt + 1) * P)

                # s = q + (emb2*wcq) @ emb1^T; u = exp(s - SHIFT) slab by slab
                u = sjt.tile([P, XL], bf16, tag="u", name=f"u{jt}")
                Zp = sjt.tile([P, NSLAB], f32, tag="Zp", name=f"Zp{jt}")
                for sl in range(NSLAB):
                    ssl = slice(sl * SLAB, (sl + 1) * SLAB)
                    sp = pss.tile([P, SLAB], f32, tag="pss", name=f"sp{jt}_{sl}")
                    nc.tensor.matmul(sp, ones_r, q_row[:, ssl].bitcast(f32r),
                                     start=True, stop=False,
                                     skip_group_check=True)
                    for hc in range(NHC):
                        nc.tensor.matmul(
                            sp, e2ts[:, hc, jsl], e1tt[:, hc, ssl],
                            start=False, stop=(hc == NHC - 1),
                            skip_group_check=True)
                    nc.scalar.activation(out=u[:, ssl], in_=sp, func=EXP,
                                         bias=negC, scale=1.0,
                                         accum_out=Zp[:, sl:sl + 1])
                nc.vector.tensor_reduce(out=Z_sb[:, jt:jt + 1], in_=Zp,
                                        axis=AXX, op=ADD)
                nc.vector.reciprocal(out=rZ_sb[:, jt:jt + 1],
                                     in_=Z_sb[:, jt:jt + 1])

                # row max for b_att: M = c + SHIFT + ln(max u)
                umax = sjt.tile([P, 1], f32, tag="umax", name=f"umax{jt}")
                nc.vector.tensor_reduce(out=umax, in_=u, axis=AXX, op=MAX)
                lnu = sjt.tile([P, 1], f32, tag="lnu", name=f"lnu{jt}")
                nc.scalar.activation(out=lnu, in_=umax, func=LN)
                nc.vector.scalar_tensor_tensor(
                    out=M_sb[:, jt:jt + 1], in0=lnu, scalar=SHIFT,
                    in1=c_sb[:, jt:jt + 1], op0=ADD, op1=ADD)

                # u^T via PE transposes, batched 4 blocks per psum tile so
                # the psum->sbuf copies are wide and don't lockstep with PE
                uT = sjt.tile([P, NIC, P], bf16, tag="uT", name=f"uT{jt}")
                for g in range(NIC // 4):
                    tp = pss.tile([P, 4, P], bf16, tag="pss", name=f"tp{jt}_{g}")
                    for k in range(4):
                        ic = g * 4 + k
                        nc.tensor.transpose(tp[:, k, :],
                                            u[:, ic * P:(ic + 1) * P], ident16)
                    nc.any.tensor_copy(out=uT[:, g * 4:(g + 1) * 4, :], in_=tp)

                # y2x_unnorm^T = emb1(natural-as-lhsT) @ uT
                yps = psy.tile([P, NHC, P], f32, tag="psy", name=f"yps{jt}")
                for hc in range(NHC):
                    for ic in range(NIC):
                        nc.tensor.matmul(
                            yps[:, hc, :], e1n[ic][:, hc * P:(hc + 1) * P],
                            uT[:, ic, :],
                            start=(ic == 0), stop=(ic == NIC - 1))

                y2xT = sjt.tile([P, NHC, P], bf16, tag="y2xT", name=f"y2xT{jt}")
                bl3 = sjt.tile([P, NHC, P], bf16, tag="bl3", name=f"bl3{jt}")
                for hc in range(NHC):
                    nc.vector.tensor_copy(out=y2xT[:, hc, :], in_=yps[:, hc, :])
                    nc.vector.tensor_mul(bl3[:, hc, :], e2tt[:, hc, jsl],
                                         y2xT[:, hc, :])

                # pass-1 reduction: [y2x; e2*y2x] @ [w2; w3]
                op1 = pso.tile([P, OUT], f32, tag="pso", name=f"op1_{jt}")
                for hc in range(NHC):
                    nc.tensor.matmul(op1, y2xT[:, hc, :], w2_sb[:, hc, :],
                                     start=(hc == 0), stop=False,
                                     skip_group_check=True)
                for hc in range(NHC):
                    nc.tensor.matmul(op1, bl3[:, hc, :], w3_sb[:, hc, :],
                                     start=False, stop=(hc == NHC - 1),
                                     skip_group_check=True)
                # out_sb = psum/Z + b_red
                nc.vector.scalar_tensor_tensor(
                    out=out_sb[:, jt, :], in0=op1, scalar=rZ_sb[:, jt:jt + 1],
                    in1=bred_bc, op0=MUL, op1=ADD)

            sjt_cm.__exit__(None, None, None)
            post_cm = tc.tile_pool(name="post", bufs=1)
            post = post_cm.__enter__()

            # ---- b_att = softmax_j(M) ----
            # global max over partitions via PE transpose + free-dim reduce,
            # then broadcast back with a K=1 matmul against a ones row.
            mx = post.tile([P, 1], f32, tag="mx")
            nc.vector.tensor_reduce(out=mx, in_=M_sb, axis=AXX, op=MAX)
            tpm = pst.tile([1, P], f32, tag="pst", name="tpm")
            nc.tensor.transpose(tpm, mx, ident32)
            mrow = post.tile([1, P], f32, tag="mrow")
            nc.vector.tensor_copy(out=mrow, in_=tpm)
            ng0 = post.tile([1, 1], f32, tag="ng0")
            nc.vector.tensor_reduce(out=ng0, in_=mrow, axis=AXX, op=MAX,
                                    negate=True)
            ngp = pst.tile([P, 1], f32, tag="pst", name="ngp")
            nc.tensor.matmul(ngp, ones32, ng0, start=True, stop=True,
                             skip_group_check=True)
            ngm = post.tile([P, 1], f32, tag="ngm")
            nc.vector.tensor_copy(out=ngm, in_=ngp)

            bexp = post.tile([P, NJT], f32, tag="bexp")
            brow = post.tile([P, 1], f32, tag="brow")
            nc.scalar.activation(out=bexp, in_=M_sb, func=EXP, bias=ngm,
                                 scale=1.0, accum_out=brow)
            tpb = pst.tile([1, P], f32, tag="pst", name="tpb")
            nc.tensor.transpose(tpb, brow, ident32)
            brw = post.tile([1, P], f32, tag="brw")
            nc.vector.tensor_copy(out=brw, in_=tpb)
            bs0 = post.tile([1, 1], f32, tag="bs0")
            nc.vector.tensor_reduce(out=bs0, in_=brw, axis=AXX, op=ADD)
            rb0 = post.tile([1, 1], f32, tag="rb0")
            nc.vector.reciprocal(rb0, bs0)
            rbp = pst.tile([P, 1], f32, tag="pst", name="rbp")
            nc.tensor.matmul(rbp, ones32, rb0, start=True, stop=True,
                             skip_group_check=True)
            rbz = post.tile([P, 1], f32, tag="rbz")
            nc.vector.tensor_copy(out=rbz, in_=rbp)
            batt = post.tile([P, NJT], bf16, tag="batt")
            nc.vector.tensor_scalar_mul(batt, bexp, rbz)

            # b_att as j-partition columns: transpose to (NJT, P), bounce
            # through a DRAM row, read back as (P, NJT) with a strided AP.
            btp = pst.tile([NJT, P], bf16, tag="pst", name="btp")
            nc.tensor.transpose(btp, batt, ident16)
            btmp = post.tile([NJT, P], bf16, tag="btmp")
            nc.vector.tensor_copy(out=btmp, in_=btp)
            scrd = dpool.tile([1, YL], bf16, tag="scrd")
            nc.sync.dma_start(out=scrd, in_=btmp)
            battjp = post.tile([P, NJT], bf16, tag="battjp")
            nc.sync.dma_start(out=battjp, in_=bass.AP(
                tensor=scrd.tensor, offset=scrd.offset, ap=[[1, P], [P, NJT]]))

            # x2y = sum_j b_att[j]*emb2[j]: PE matmuls with the b_att column
            # as a 1-wide stationary operand against natural emb2 chunks
            # re-read from DRAM (bf16), accumulated over j chunks.
            e2nt = [post.tile([P, H], bf16, tag=f"e2n{jc}", name=f"e2n{jc}")
                    for jc in range(NJT)]
            for jc in range(NJT):
                nc.sync.dma_start(out=e2nt[jc], in_=emb2_d[jc * P:(jc + 1) * P, :])
            x2p = psy.tile([1, H], f32, tag="psy", name="x2p")
            for hsl in (slice(0, 512), slice(512, H)):
                for jc in range(NJT):
                    nc.tensor.matmul(
                        x2p[:, hsl], battjp[:, jc:jc + 1], e2nt[jc][:, hsl],
                        start=(jc == 0), stop=(jc == NJT - 1),
                        skip_group_check=True)
            x2row = post.tile([1, H], f32, tag="x2row")
            nc.any.tensor_copy(out=x2row, in_=x2p)
            x2d = dpool.tile([1, H], f32, tag="x2d")
            nc.sync.dma_start(out=x2d, in_=x2row)
            x2yT = post.tile([P, NHC], f32, tag="x2yT")
            nc.sync.dma_start(out=x2yT, in_=bass.AP(
                tensor=x2d.tensor, offset=x2d.offset, ap=[[1, P], [P, NHC]]))

            # w14' = w1 + x2y*w4
            w14 = res.tile([P, NHC, OUT], bf16, tag="w14")
            for hc in range(NHC):
                nc.vector.scalar_tensor_tensor(
                    out=w14[:, hc, :], in0=w4_sb[:, hc, :],
                    scalar=x2yT[:, hc:hc + 1], in1=w1_sb[:, hc, :],
                    op0=MUL, op1=ADD)

            # ---- pass 2: out += emb2 @ w14' ----
            for jt in range(NJT):
                jsl = slice(jt * P, (jt + 1) * P)
                op2 = pso.tile([P, OUT], f32, tag="pso", name=f"op2_{jt}")
                for hc in range(NHC):
                    nc.tensor.matmul(op2, e2tt[:, hc, jsl], w14[:, hc, :],
                                     start=(hc == 0), stop=(hc == NHC - 1),
                                     skip_group_check=True)
                fin = stage.tile([P, OUT], f32, tag="fin", name=f"fin{jt}")
                nc.vector.tensor_add(fin, op2, out_sb[:, jt, :])
                nc.sync.dma_start(out=out_d[jsl, :], in_=fin)
            post_cm.__exit__(None, None, None)

    return nc


def _get_nc(drain_fix=True):
    if "nc" not in _CACHE:
        _CACHE["nc"] = _build()
    if drain_fix and not _CACHE.get("drain_fixed"):
        import concourse.mybir as mybir
        _fix_waits(_CACHE["nc"], mybir, max_waits=1)
        _CACHE["drain_fixed"] = True
    return _CACHE["nc"]


def kernel(emb1, emb2, w_c, b_c, w_q, b_q, w_cq, b_cq, w_red, b_red):
    from concourse.bass_utils import run_bass_kernel_spmd

    nc = _get_nc()
    bf = ml_dtypes.bfloat16

    emb1 = np.ascontiguousarray(np.asarray(emb1, dtype=np.float32).astype(bf))
    emb2 = np.ascontiguousarray(np.asarray(emb2, dtype=np.float32).astype(bf))
    w_red = np.asarray(w_red, dtype=np.float32)

    # b_c, b_q, b_cq cancel exactly in both softmaxes (per-row/col consts).
    wc = np.ascontiguousarray(np.asarray(w_c, np.float32).reshape(NHC, P).T.astype(bf))
    wq = np.ascontiguousarray(np.asarray(w_q, np.float32).reshape(NHC, P).T.astype(bf))
    wcq = np.ascontiguousarray(np.asarray(w_cq, np.float32).reshape(NHC, P).T)
    w1 = np.ascontiguousarray(w_red[0:H])
    w2 = np.ascontiguousarray(w_red[H:2 * H].astype(bf))
    w3 = np.ascontiguousarray(w_red[2 * H:3 * H].astype(bf))
    w4 = np.ascontiguousarray(w_red[3 * H:4 * H])
    bred = np.ascontiguousarray(np.asarray(b_red, np.float32).reshape(1, OUT))

    in_maps = []
    for b in range(NCORES):
        in_maps.append({
            "emb1": emb1[b], "emb2": emb2[b],
            "wc": wc, "wq": wq, "wcq": wcq,
            "w1": w1, "w2": w2, "w3": w3, "w4": w4, "bred": bred,
        })
    res = run_bass_kernel_spmd(nc, in_maps, core_ids=list(range(NCORES)))
    return np.stack([res.results[i]["out"] for i in range(NCORES)], axis=0)



# revision 11
# speedup vs baseline: 1.6256x; 1.6256x over previous
"""BiDAF attention-flow kernel for Trainium2 (8 NeuronCores, data-parallel over batch).

Per core (one batch element):
  s[j,i]   = c[j] + q[i] + sum_h w_cq[h]*emb2[j,h]*emb1[i,h]
  a        = softmax_i(s)          (c[j] drops out of the row softmax)
  y2x      = a @ emb1
  b_att    = softmax_j(max_i s)
  x2y      = sum_j b_att[j]*emb2[j]
  out      = [emb2, y2x, emb2*y2x, emb2*x2y] @ w_red + b_red

v2 structure:
  - Inputs land in 2 big DMAs per embedding (1.5 MB each) straight into the
    resident natural tiles; weights are packed into 2 big + 3 small DMAs.
    This amortizes the ~2us fixed cost per dma_start that serialized v1.
  - u^T is produced by the DMA xbar transpose (one dma per j-tile) instead of
    16 PE transposes, freeing the Tensor engine for matmuls.
  - y2x is batched over PAIRS of j-tiles so the moving operand is 256 wide
    (halves the LDWEIGHTS pressure of the N=128 version).
  - b_att column layout == its natural [P, NJT] layout (the v1 DRAM bounce
    was an identity); x2y/c reshapes use tiny PE transposes, not DRAM.
  - emb2 natural stays resident for the x2y tail (no DRAM re-read).
  - Fixed exp shift (s - SHIFT); row max recovered as SHIFT + ln(max u).
  - pass1 computes [y2x; emb2*y2x] @ [w2; w3] + per-row 1/Z on the psum;
    pass2 adds emb2 @ (w1 + x2y*w4) and streams out in 4-tile DMA batches.
"""

import numpy as np
import ml_dtypes

P = 128
XL = 2048
YL = 2048
H = 768
OUT = 300
NJT = YL // P   # 16 j tiles
NIC = XL // P   # 16 i chunks
NHC = H // P    # 6 h chunks
SLAB = 512
NSLAB = XL // SLAB  # 4
NPAIR = NJT // 2
NCORES = 8
SHIFT = 10.0    # fixed exp shift; |s| stays well below this + fp range
NWARM = 60

_CACHE = {}
_PHASE_MARKS = []  # (first_unused_id, tag) checkpoints for trace attribution


def _fix_waits(nc, mybir, max_waits=1):
    """This walrus build rejects >1 sync wait per instruction.

    Pass 1: drop waits that are transitively implied by another wait on the
    same instruction (happens-before over per-engine / per-DMA-queue in-order
    streams plus wait edges).  Pass 2: hoist remaining extra waits onto
    same-engine NoOps inserted right before the instruction (for an in-order
    engine this blocks identically; DMA triggers are all on SP here and their
    awaited DMAs are always triggered earlier, so no cycles arise).
    """
    from collections import defaultdict

    blocks = [bb for f in nc.m.functions for bb in f.blocks]
    insts = [ins for bb in blocks for ins in bb.instructions]

    dma_types = ("InstDMACopy", "InstDmaTransposeAnt")
    eng_stream = defaultdict(list)
    queue_stream = defaultdict(list)
    sem_events = defaultdict(list)
    cum = defaultdict(int)
    for i, ins in enumerate(insts):
        eng_stream[str(ins.engine)].append(i)
        si = ins.sync_info
        if si and si.on_update:
            for u in si.on_update:
                cum[u.id] += u.update_value
                sem_events[u.id].append((cum[u.id], i))
                if type(ins).__name__ in dma_types:
                    queue_stream[u.id].append(i)

    def achiever(sem_id, val):
        for cv, i in sem_events.get(sem_id, []):
            if cv >= val:
                return i
        return None

    eng_pos, q_pos = {}, {}
    for e, lst in eng_stream.items():
        for k, i in enumerate(lst):
            eng_pos[i] = (e, k)
    for s, lst in queue_stream.items():
        for k, i in enumerate(lst):
            q_pos[i] = (s, k)

    memo = {}

    def implied(i):
        if i in memo:
            return memo[i]
        memo[i] = set()
        out = {i}
        ins = insts[i]
        if i in q_pos:
            s, k = q_pos[i]
            if k > 0:
                out |= implied(queue_stream[s][k - 1])
        e, k = eng_pos[i]
        j = k - 1
        while j >= 0:
            p = eng_stream[e][j]
            if type(insts[p]).__name__ in dma_types:
                j -= 1
                continue
            out |= implied(p)
            break
        si = ins.sync_info
        if si and si.on_wait:
            for w in si.on_wait:
                a = achiever(w.id, w.wait_value)
                if a is not None:
                    out |= implied(a)
        memo[i] = out
        return out

    # pass 1: redundancy elimination
    for i, ins in enumerate(insts):
        si = ins.sync_info
        if not (si and si.on_wait and len(si.on_wait) > max_waits):
            continue
        waits = list(si.on_wait)
        ach = [(w, achiever(w.id, w.wait_value)) for w in waits]
        keep = []
        for wi, (w, a) in enumerate(ach):
            red = False
            if a is not None:
                for wj, (w2, a2) in enumerate(ach):
                    if wi != wj and a2 is not None and a != a2 and a in implied(a2):
                        red = True
                        break
            if not red:
                keep.append(w)
        si.on_wait = keep

    # pass 2: hoist extras onto same-engine NoOps
    k = 0
    for bb in blocks:
        lst = bb.instructions
        i = 0
        while i < len(lst):
            ins = lst[i]
            si = ins.sync_info
            if si and si.on_wait and len(si.on_wait) > max_waits:
                waits = list(si.on_wait)
                extra, keep = waits[:-max_waits], waits[-max_waits:]
                si.on_wait = keep
                nops = []
                for w in extra:
                    nop = mybir.InstNoOp(name=f"I-waitfix-{k}", ins=[], outs=[])
                    k += 1
                    nop.engine = ins.engine
                    nop.sync_info = mybir.SyncInfo(on_wait=[w], on_update=[])
                    nops.append(nop)
                lst[i:i] = nops
                i += len(nops)
            i += 1


def _build():
    import concourse.bass as bass
    import concourse.tile as tile
    import concourse.mybir as mybir
    from concourse.masks import make_identity

    f32 = mybir.dt.float32
    f32r = mybir.dt.float32r
    bf16 = mybir.dt.bfloat16
    MUL = mybir.AluOpType.mult
    ADD = mybir.AluOpType.add
    MAX = mybir.AluOpType.max
    EXP = mybir.ActivationFunctionType.Exp
    LN = mybir.ActivationFunctionType.Ln
    AXX = mybir.AxisListType.X

    nc = bass.Bass("TRN2", target_bir_lowering=False, debug=False,
                   num_devices=NCORES)

    _PHASE_MARKS.clear()

    def mark(tag):
        _PHASE_MARKS.append((nc.next_id(), tag))

    emb1_d = nc.dram_tensor("emb1", [XL, H], bf16, kind="ExternalInput")
    emb2_d = nc.dram_tensor("emb2", [YL, H], bf16, kind="ExternalInput")
    wsm_d = nc.dram_tensor("wsm", [P, 2 * NHC], bf16, kind="ExternalInput")
    wcq_d = nc.dram_tensor("wcq", [P, NHC], f32, kind="ExternalInput")
    wrf_d = nc.dram_tensor("wrf", [P, NHC, 2 * OUT], f32, kind="ExternalInput")
    wrb_d = nc.dram_tensor("wrb", [P, NHC, 2 * OUT], bf16, kind="ExternalInput")
    bred_d = nc.dram_tensor("bred", [1, OUT], f32, kind="ExternalInput")
    out_d = nc.dram_tensor("out", [YL, OUT], f32, kind="ExternalOutput")

    # DRAM views with 128-row partition folding: [p, chunk, h]
    e1r = emb1_d.ap().rearrange("(c p) h -> p c h", p=P)
    e2r = emb2_d.ap().rearrange("(c p) h -> p c h", p=P)
    outr = out_d.ap().rearrange("(c p) o -> p c o", p=P)

    with tile.TileContext(nc) as tc:
        with (
            tc.tile_pool(name="res", bufs=1) as res,        # resident data
            tc.tile_pool(name="small", bufs=1) as small,    # stats etc
            tc.tile_pool(name="upool", bufs=2) as upool,    # u tiles
            tc.tile_pool(name="utp", bufs=2) as utp,        # uT pair tiles
            tc.tile_pool(name="ypool", bufs=2) as ypool,    # y2xT/bl3 pair tiles
            tc.tile_pool(name="pss", bufs=2, space="PSUM") as pss,
            tc.tile_pool(name="psy", bufs=1, space="PSUM") as psy,
            tc.tile_pool(name="pso", bufs=2, space="PSUM") as pso,
        ):
            # ---- constants ----
            ident16 = res.tile([P, P], bf16, tag="ident16")
            make_identity(nc, ident16)
            ident32 = res.tile([P, P], f32, tag="ident32")
            make_identity(nc, ident32)
            ones32 = res.tile([1, P], f32, tag="ones32")
            nc.vector.memset(ones32, 1.0)
            ones_bf = res.tile([1, P], bf16, tag="ones_bf")
            nc.vector.tensor_copy(out=ones_bf, in_=ones32)
            negC = res.tile([P, 1], f32, tag="negC")
            nc.vector.memset(negC, -SHIFT)

            mark("warm")
            # PE warm-up with REAL matmuls (transpose-mode doesn't engage the
            # HAM activity monitor); keeps the clock at 2.4 GHz while the
            # input DMAs stream in.  Results are discarded.
            for wk in range(NWARM):
                wps = pss.tile([P, P], f32, tag="pss", name=f"warm{wk}")
                nc.tensor.matmul(wps, ident16, ident16, start=True, stop=True,
                                 skip_group_check=True)

            mark("wload")
            # ---- weights + embeddings: few big DMAs ----
            wsm_sb = res.tile([P, 2 * NHC], bf16, tag="wsm")
            nc.sync.dma_start(out=wsm_sb, in_=wsm_d[:])
            wc_sb = wsm_sb[:, 0:NHC]
            wq_sb = wsm_sb[:, NHC:2 * NHC]
            wcq_sb = res.tile([P, NHC], f32, tag="wcq")
            nc.sync.dma_start(out=wcq_sb, in_=wcq_d[:])
            bred_bc = res.tile([P, OUT], f32, tag="bred_bc")
            _bap = bred_d.ap()
            nc.sync.dma_start(out=bred_bc, in_=bass.AP(
                tensor=_bap.tensor, offset=_bap.offset,
                ap=[[0, P]] + list(_bap.ap[1:])))

            e1n = res.tile([P, NIC, H], bf16, tag="e1n")
            e2n = res.tile([P, NJT, H], bf16, tag="e2n")
            nc.sync.dma_start(out=e1n[:, 0:8, :], in_=e1r[:, 0:8, :])
            nc.sync.dma_start(out=e1n[:, 8:16, :], in_=e1r[:, 8:16, :])
            nc.scalar.dma_start(out=e2n[:, 0:8, :], in_=e2r[:, 0:8, :])
            nc.scalar.dma_start(out=e2n[:, 8:16, :], in_=e2r[:, 8:16, :])

            wrf_sb = res.tile([P, NHC, 2 * OUT], f32, tag="wrf")
            nc.scalar.dma_start(out=wrf_sb, in_=wrf_d[:])
            wrb_sb = res.tile([P, NHC, 2 * OUT], bf16, tag="wrb")
            nc.scalar.dma_start(out=wrb_sb, in_=wrb_d[:])
            w1_sb = wrf_sb[:, :, 0:OUT]
            w4_sb = wrf_sb[:, :, OUT:2 * OUT]
            w2_sb = wrb_sb[:, :, 0:OUT]
            w3_sb = wrb_sb[:, :, OUT:2 * OUT]

            mark("eload")
            # ---- transposed layouts via PE (hidden under input DMA) ----
            e1tt = res.tile([P, NHC, XL], bf16, tag="e1tt")
            e2tt = res.tile([P, NHC, YL], bf16, tag="e2tt")

            for g in range(4):          # groups of 4 i-chunks
                for hc in range(NHC):
                    tp = pss.tile([P, 4, P], bf16, tag="pss",
                                  name=f"e1tp{g}_{hc}")
                    for k in range(4):
                        ic = g * 4 + k
                        nc.tensor.transpose(
                            tp[:, k, :],
                            e1n[:, ic, hc * P:(hc + 1) * P], ident16)
                    nc.any.tensor_copy(
                        out=e1tt[:, hc, g * 512:(g + 1) * 512], in_=tp)

            mark("qrow")
            # q_row = emb1 @ w_q as a (1, XL) row
            q_row = small.tile([1, XL], bf16, tag="q_row")
            for sl in range(NSLAB):
                ssl = slice(sl * SLAB, (sl + 1) * SLAB)
                qp = pss.tile([1, SLAB], f32, tag="pss", name=f"qp{sl}")
                for hc in range(NHC):
                    nc.tensor.matmul(
                        qp, wq_sb[:, hc:hc + 1], e1tt[:, hc, ssl],
                        start=(hc == 0), stop=(hc == NHC - 1),
                        skip_group_check=True)
                nc.any.tensor_copy(out=q_row[:, ssl], in_=qp)

            mark("e1scale")
            # fold w_cq into the s-matmul moving operand: e1tt *= wcq[h]
            # (q_row is already computed from the unscaled e1tt)
            for hc in range(NHC):
                for g in range(4):
                    gsl = slice(g * 512, (g + 1) * 512)
                    nc.vector.tensor_scalar_mul(
                        e1tt[:, hc, gsl], e1tt[:, hc, gsl],
                        wcq_sb[:, hc:hc + 1])

            mark("e2load")
            for g in range(4):          # groups of 4 j-chunks
                for hc in range(NHC):
                    tp = pss.tile([P, 4, P], bf16, tag="pss",
                                  name=f"e2tp{g}_{hc}")
                    for k in range(4):
                        jc = g * 4 + k
                        nc.tensor.transpose(
                            tp[:, k, :],
                            e2n[:, jc, hc * P:(hc + 1) * P], ident16)
                    nc.any.tensor_copy(
                        out=e2tt[:, hc, g * 512:(g + 1) * 512], in_=tp)

            mark("crow")
            # c_row = emb2 @ w_c as a (1, YL) row, then 16 tiny PE transposes
            # into per-partition columns c_sb[:, jt]
            c_rowf = small.tile([1, YL], bf16, tag="c_rowf")
            for sl in range(NSLAB):
                ssl = slice(sl * SLAB, (sl + 1) * SLAB)
                cp = pss.tile([1, SLAB], f32, tag="pss", name=f"cp{sl}")
                for hc in range(NHC):
                    nc.tensor.matmul(
                        cp, wc_sb[:, hc:hc + 1], e2tt[:, hc, ssl],
                        start=(hc == 0), stop=(hc == NHC - 1),
                        skip_group_check=True)
                nc.any.tensor_copy(out=c_rowf[:, ssl], in_=cp)
            c_sb = small.tile([P, NJT], f32, tag="c_sb")
            for jt in range(NJT):
                ctp = pss.tile([P, 1], bf16, tag="pss", name=f"ctp{jt}")
                nc.tensor.transpose(
                    ctp, c_rowf[:, jt * P:(jt + 1) * P], ident16[0:1, 0:1])
                nc.any.tensor_copy(out=c_sb[:, jt:jt + 1], in_=ctp)

            # ---- stats tiles ----
            M_sb = small.tile([P, NJT], f32, tag="M")
            Z_sb = small.tile([P, NJT], f32, tag="Z")
            rZ_sb = small.tile([P, NJT], f32, tag="rZ")
            out_sb = res.tile([P, NJT, OUT], f32, tag="out_sb")

            # ---- main loop over j-tile pairs ----
            # per pair g: emit s/exp/xbar for jt=2g,2g+1; then y2x for pair
            # g-1; then pass1 for pair g-1's two j tiles.
            pair_state = {}

            def emit_s(jt):
                mark("jt_s")
                jsl = slice(jt * P, (jt + 1) * P)
                u = upool.tile([P, XL], bf16, tag="u", name=f"u{jt}")
                Zp = upool.tile([P, NSLAB], f32, tag="Zp", name=f"Zp{jt}")
                for sl in range(NSLAB):
                    ssl = slice(sl * SLAB, (sl + 1) * SLAB)
                    sp = pss.tile([P, SLAB], f32, tag="pss",
                                  name=f"sp{jt}_{sl}")
                    nc.tensor.matmul(sp, ones_bf, q_row[:, ssl],
                                     start=True, stop=False,
                                     skip_group_check=True)
                    for hc in range(NHC):
                        nc.tensor.matmul(
                            sp, e2tt[:, hc, jsl], e1tt[:, hc, ssl],
                            start=False, stop=(hc == NHC - 1),
                            skip_group_check=True)
                    nc.scalar.activation(out=u[:, ssl], in_=sp, func=EXP,
                                         bias=negC, scale=1.0,
                                         accum_out=Zp[:, sl:sl + 1])
                mark("jt_stats")
                nc.vector.tensor_reduce(out=Z_sb[:, jt:jt + 1], in_=Zp,
                                        axis=AXX, op=ADD)
                nc.vector.reciprocal(out=rZ_sb[:, jt:jt + 1],
                                     in_=Z_sb[:, jt:jt + 1])
                umax = upool.tile([P, 1], f32, tag="umax", name=f"umax{jt}")
                nc.vector.tensor_reduce(out=umax, in_=u, axis=AXX, op=MAX)
                lnu = upool.tile([P, 1], f32, tag="lnu", name=f"lnu{jt}")
                nc.scalar.activation(out=lnu, in_=umax, func=LN)
                nc.vector.scalar_tensor_tensor(
                    out=M_sb[:, jt:jt + 1], in0=lnu, scalar=SHIFT,
                    in1=c_sb[:, jt:jt + 1], op0=ADD, op1=ADD)
                return u

            def emit_xbar(g, ulo, uhi):
                mark("jt_uT")
                # u^T for the pair via the DMA transpose crossbar:
                # uT2[p, ic, jj] = u[jj, ic*128+p]
                uT2 = utp.tile([P, NIC, 2 * P], bf16, tag="uT2",
                               name=f"uT2_{g}")
                nc.sync.dma_start(out=uT2[:, :, 0:P], in_=ulo,
                                  transpose=True)
                nc.sync.dma_start(out=uT2[:, :, P:2 * P], in_=uhi,
                                  transpose=True)
                return uT2

            def emit_y2x(g):
                mark("jt_y2x")
                uT2 = pair_state[g]["uT2"]
                psl = slice(2 * g * P, (2 * g + 2) * P)
                yps = psy.tile([P, NHC, 2 * P], f32, tag="psy",
                               name=f"yps{g}")
                for hc in range(NHC):
                    for ic in range(NIC):
                        nc.tensor.matmul(
                            yps[:, hc, :],
                            e1n[:, ic, hc * P:(hc + 1) * P],
                            uT2[:, ic, :],
                            start=(ic == 0), stop=(ic == NIC - 1))
                mark("jt_y2xc")
                y2xT = ypool.tile([P, NHC, 2 * P], bf16, tag="y2xT",
                                  name=f"y2xT{g}")
                bl3 = ypool.tile([P, NHC, 2 * P], bf16, tag="bl3",
                                 name=f"bl3{g}")
                for hc in range(NHC):
                    nc.any.tensor_copy(out=y2xT[:, hc, :], in_=yps[:, hc, :])
                    nc.vector.tensor_mul(bl3[:, hc, :], e2tt[:, hc, psl],
                                         y2xT[:, hc, :])
                pair_state[g]["y2xT"] = y2xT
                pair_state[g]["bl3"] = bl3

            def emit_pass1(g):
                mark("jt_pass1")
                y2xT = pair_state[g]["y2xT"]
                bl3 = pair_state[g]["bl3"]
                for half in range(2):
                    jt = 2 * g + half
                    hsl = slice(half * P, (half + 1) * P)
                    op1 = pso.tile([P, OUT], f32, tag="pso",
                                   name=f"op1_{jt}")
                    for hc in range(NHC):
                        nc.tensor.matmul(op1, y2xT[:, hc, hsl],
                                         w2_sb[:, hc, :],
                                         start=(hc == 0), stop=False,
                                         skip_group_check=True)
                    for hc in range(NHC):
                        nc.tensor.matmul(op1, bl3[:, hc, hsl],
                                         w3_sb[:, hc, :],
                                         start=False, stop=(hc == NHC - 1),
                                         skip_group_check=True)
                    nc.vector.scalar_tensor_tensor(
                        out=out_sb[:, jt, :], in0=op1,
                        scalar=rZ_sb[:, jt:jt + 1],
                        in1=bred_bc, op0=MUL, op1=ADD)
                del pair_state[g]

            for g in range(NPAIR):
                ulo = emit_s(2 * g)
                uhi = emit_s(2 * g + 1)
                pair_state[g] = {"uT2": emit_xbar(g, ulo, uhi)}
                if g >= 1:
                    emit_y2x(g - 1)
                if g >= 2:
                    emit_pass1(g - 2)
            emit_y2x(NPAIR - 1)
            emit_pass1(NPAIR - 2)
            emit_pass1(NPAIR - 1)

            mark("batt")
            post_cm = tc.tile_pool(name="post", bufs=1)
            post = post_cm.__enter__()

            # ---- b_att = softmax_j(M) ----
            # global max over partitions via PE transpose + free-dim reduce,
            # then broadcast back with a K=1 matmul against a ones row.
            mx = post.tile([P, 1], f32, tag="mx")
            nc.vector.tensor_reduce(out=mx, in_=M_sb, axis=AXX, op=MAX)
            tpm = pss.tile([1, P], f32, tag="pss", name="tpm")
            nc.tensor.transpose(tpm, mx, ident32)
            mrow = post.tile([1, P], f32, tag="mrow")
            nc.vector.tensor_copy(out=mrow, in_=tpm)
            ng0 = post.tile([1, 1], f32, tag="ng0")
            nc.vector.tensor_reduce(out=ng0, in_=mrow, axis=AXX, op=MAX,
                                    negate=True)
            ngp = pss.tile([P, 1], f32, tag="pss", name="ngp")
            nc.tensor.matmul(ngp, ones32, ng0, start=True, stop=True,
                             skip_group_check=True)
            ngm = post.tile([P, 1], f32, tag="ngm")
            nc.vector.tensor_copy(out=ngm, in_=ngp)

            bexp = post.tile([P, NJT], f32, tag="bexp")
            brow = post.tile([P, 1], f32, tag="brow")
            nc.scalar.activation(out=bexp, in_=M_sb, func=EXP, bias=ngm,
                                 scale=1.0, accum_out=brow)
            tpb = pss.tile([1, P], f32, tag="pss", name="tpb")
            nc.tensor.transpose(tpb, brow, ident32)
            brw = post.tile([1, P], f32, tag="brw")
            nc.vector.tensor_copy(out=brw, in_=tpb)
            bs0 = post.tile([1, 1], f32, tag="bs0")
            nc.vector.tensor_reduce(out=bs0, in_=brw, axis=AXX, op=ADD)
            rb0 = post.tile([1, 1], f32, tag="rb0")
            nc.vector.reciprocal(rb0, bs0)
            rbp = pss.tile([P, 1], f32, tag="pss", name="rbp")
            nc.tensor.matmul(rbp, ones32, rb0, start=True, stop=True,
                             skip_group_check=True)
            rbz = post.tile([P, 1], f32, tag="rbz")
            nc.vector.tensor_copy(out=rbz, in_=rbp)
            # batt[p, jc] = b_att[jc*128+p] -- already the per-j-chunk column
            # layout needed as x2y matmul stationary.
            batt = post.tile([P, NJT], bf16, tag="batt")
            nc.vector.tensor_scalar_mul(batt, bexp, rbz)

            mark("x2y")
            # x2y = sum_j b_att[j]*emb2[j] via PE on resident natural emb2
            x2p = psy.tile([1, H], f32, tag="psy", name="x2p")
            for hsl in (slice(0, 512), slice(512, H)):
                for jc in range(NJT):
                    nc.tensor.matmul(
                        x2p[:, hsl], batt[:, jc:jc + 1], e2n[:, jc, hsl],
                        start=(jc == 0), stop=(jc == NJT - 1),
                        skip_group_check=True)
            x2row = post.tile([1, H], f32, tag="x2row")
            nc.any.tensor_copy(out=x2row, in_=x2p)
            # spread to partitions: 6 tiny PE transposes [1,128] -> [128,1]
            x2yT = post.tile([P, NHC], f32, tag="x2yT")
            for hc in range(NHC):
                xtp = pss.tile([P, 1], f32, tag="pss", name=f"xtp{hc}")
                nc.tensor.transpose(
                    xtp, x2row[:, hc * P:(hc + 1) * P], ident32[0:1, 0:1])
                nc.any.tensor_copy(out=x2yT[:, hc:hc + 1], in_=xtp)

            mark("w14")
            # w14' = w1 + x2y*w4
            w14 = res.tile([P, NHC, OUT], bf16, tag="w14")
            for hc in range(NHC):
                nc.vector.scalar_tensor_tensor(
                    out=w14[:, hc, :], in0=w4_sb[:, hc, :],
                    scalar=x2yT[:, hc:hc + 1], in1=w1_sb[:, hc, :],
                    op0=MUL, op1=ADD)

            mark("pass2")
            # ---- pass 2: out += emb2 @ w14', 4-j-tile batched output DMAs
            for qt in range(NJT // 4):
                for k in range(4):
                    jt = qt * 4 + k
                    jsl = slice(jt * P, (jt + 1) * P)
                    op2 = pso.tile([P, OUT], f32, tag="pso",
                                   name=f"op2_{jt}")
                    for hc in range(NHC):
                        nc.tensor.matmul(op2, e2tt[:, hc, jsl],
                                         w14[:, hc, :],
                                         start=(hc == 0), stop=(hc == NHC - 1),
                                         skip_group_check=True)
                    nc.vector.tensor_add(out_sb[:, jt, :], op2,
                                         out_sb[:, jt, :])
                nc.sync.dma_start(out=outr[:, qt * 4:(qt + 1) * 4, :],
                                  in_=out_sb[:, qt * 4:(qt + 1) * 4, :])
            post_cm.__exit__(None, None, None)

    return nc


def _get_nc(drain_fix=True):
    if "nc" not in _CACHE:
        _CACHE["nc"] = _build()
    if drain_fix and not _CACHE.get("drain_fixed"):
        import concourse.mybir as mybir
        _fix_waits(_CACHE["nc"], mybir, max_waits=1)
        _CACHE["drain_fixed"] = True
    return _CACHE["nc"]


def _prep_weights(w_c, w_q, w_cq, w_red, b_red):
    bf = ml_dtypes.bfloat16
    w_red = np.asarray(w_red, dtype=np.float32)
    wc = np.asarray(w_c, np.float32).reshape(NHC, P).T
    wq = np.asarray(w_q, np.float32).reshape(NHC, P).T
    wsm = np.ascontiguousarray(
        np.concatenate([wc, wq], axis=1).astype(bf))
    wcq = np.ascontiguousarray(np.asarray(w_cq, np.float32).reshape(NHC, P).T)

    # wrf[p, hc, 0:OUT] = w1[hc*P+p]; wrf[p, hc, OUT:] = w4[hc*P+p]
    w1 = w_red[0:H].reshape(NHC, P, OUT)
    w2 = w_red[H:2 * H].reshape(NHC, P, OUT)
    w3 = w_red[2 * H:3 * H].reshape(NHC, P, OUT)
    w4 = w_red[3 * H:4 * H].reshape(NHC, P, OUT)
    wrf = np.ascontiguousarray(
        np.concatenate([w1, w4], axis=2).transpose(1, 0, 2))
    wrb = np.ascontiguousarray(
        np.concatenate([w2, w3], axis=2).transpose(1, 0, 2).astype(bf))
    bred = np.ascontiguousarray(np.asarray(b_red, np.float32).reshape(1, OUT))
    return {"wsm": wsm, "wcq": wcq, "wrf": wrf, "wrb": wrb, "bred": bred}


def kernel(emb1, emb2, w_c, b_c, w_q, b_q, w_cq, b_cq, w_red, b_red):
    from concourse.bass_utils import run_bass_kernel_spmd

    nc = _get_nc()
    bf = ml_dtypes.bfloat16

    emb1 = np.ascontiguousarray(np.asarray(emb1, dtype=np.float32).astype(bf))
    emb2 = np.ascontiguousarray(np.asarray(emb2, dtype=np.float32).astype(bf))

    # b_c, b_q, b_cq cancel exactly in both softmaxes (per-row/col consts).
    wmap = _prep_weights(w_c, w_q, w_cq, w_red, b_red)

    in_maps = []
    for b in range(NCORES):
        in_maps.append({"emb1": emb1[b], "emb2": emb2[b], **wmap})
    res = run_bass_kernel_spmd(nc, in_maps, core_ids=list(range(NCORES)))
    return np.stack([res.results[i]["out"] for i in range(NCORES)], axis=0)
